# revision 20
# baseline (speedup 1.0000x reference)
"""Bbox regression loss (smooth-L1 over gathered bbox deltas) on 8 TRN2 cores.

The loss gathers 4 scalars per (batch, gt-box) from each FPN level's dense
prediction tensor, applies smooth-L1 against the gt deltas, and reduces to
two scalars (weighted loss sum, valid-box count).  Only 3 x 2 x 128 x 4 =
3072 elements of the ~92MB of predictions are ever read, so the kernel is
built around one on-device dma_gather rather than streaming.

Sharding: core c handles (b = c//4, k = c%4) where k indexes the 4 bbox
coordinate channels (channel group k*A:(k+1)*A of the 4*A=12 channel dim).
Each core receives exactly 1/8 of every prediction tensor (concatenated
into one row table), computes its partial (loss, weight) fully on device,
and the host sums the 8 partials.

Device pipeline per core:
  1. coord math -> 512B-row indices for all 3 levels in the
     16-partition-wrapped int16 layout dma_gather wants.  The host ships
     stride-premultiplied coordinate terms (incl. the concat-table row
     base as a 5th term), so the device only clamps, 5-term-reduces and
     shifts.  Clamping the premultiplied anchor term is exact because only
     the anchor carries the -1 sentinel and the strides are positive.
  2. one dma_gather fetches the 384 rows -> [128(m), 3(level), 128] f32
  3. fused scalar_tensor_tensor one-hot select (iota==rem)*g with
     per-partition accumulate -> pred[m,l]
  4. smooth-L1 via the huber identity 0.5*(d^2 - relu(|d|-1)^2) (the 0.5
     folded into the host-side weight), validity masking, one matmul
     partition-reduction, per-level active mask applied on partition 0
"""

import os

import numpy as np

try:  # persistent XLA/NEFF compile cache across processes
    import jax

    os.makedirs("/tmp/jax_pcache", exist_ok=True)
    jax.config.update("jax_compilation_cache_dir", "/tmp/jax_pcache")
    jax.config.update("jax_persistent_cache_min_compile_time_secs", 0.0)
    jax.config.update("jax_persistent_cache_min_entry_size_bytes", 0)
except Exception:
    pass

import concourse.bacc as bacc
import concourse.bass as bass
import concourse.tile as tile
from concourse import mybir
from concourse.bass_utils import run_bass_kernel_spmd

A = 3                       # anchors per level
M = 128                     # gt entries per sample
GRIDS = (96, 48, 24)        # level l grid; level l uses coord/diff index 2-l
LOSS_W = (1.0, 1.0, 1.0, 0.1)
ROW = 128                   # f32 elements per gather row (512B)
NLVL = 3
NIDX = NLVL * M             # 384 gathered rows per core
V = tuple(A * g * g * g // ROW for g in GRIDS)      # (20736, 2592, 324)
VBASE = (0, V[0], V[0] + V[1])
VTOT = sum(V)               # 23652 rows < int16 max
N_CORES = 8

F32 = mybir.dt.float32
I32 = mybir.dt.int32
I16 = mybir.dt.int16
Alu = mybir.AluOpType

# auxp (int32, [128, 135]): stride-premultiplied coord terms, 27 groups x 5:
#   groups 0-2   natural layout (partition = m), 5th term 0
#   groups 3-26  wrapped layout (l,q): partition = m%16, m = q*16+p%16,
#                5th term = VBASE[l]*ROW (element base of level l's rows)
NCOORD = 135
# auxf (f32, [128, 132]): iota128 | gts(3) | ones
NF_COLS = ROW + 4


def _build_bass() -> bass.Bass:
    nc = bacc.Bacc(
        "TRN2", target_bir_lowering=False, debug=False, num_devices=N_CORES
    )
    tab = nc.dram_tensor("tab", [VTOT, ROW], F32, kind="ExternalInput")
    auxp = nc.dram_tensor("auxp", [M, NCOORD], I32, kind="ExternalInput")
    auxf = nc.dram_tensor("auxf", [M, NF_COLS], F32, kind="ExternalInput")
    out = nc.dram_tensor("partial", [1, 6], F32, kind="ExternalOutput")

    with tile.TileContext(nc) as tc:
        with (
            tc.tile_pool(name="sb", bufs=1) as sb,
            tc.tile_pool(name="ps", bufs=1, space="PSUM") as ps,
        ):
            ti = sb.tile([M, NCOORD], I32)
            nc.sync.dma_start(out=ti[:], in_=auxp[:])
            tf = sb.tile([M, NF_COLS], F32)
            nc.sync.dma_start(out=tf[:], in_=auxf[:])
            iota = tf[:, 0:ROW]
            gts = tf[:, ROW : ROW + 3]
            onec = tf[:, ROW + 3 : ROW + 4]

            # flat[., c] = sum of premultiplied terms (no clamp: only the
            # anchor term of invalid entries is negative; the row index is
            # clamped at 0 in the shift below and those entries are masked)
            flat = sb.tile([M, 27], I32)
            with nc.allow_low_precision(reason="exact int32 index arithmetic"):
                nc.vector.tensor_reduce(
                    flat[:],
                    ti[:].rearrange("p (c f) -> p c f", f=5),
                    axis=mybir.AxisListType.X,
                    op=Alu.add,
                )

            # wrapped row indices (int16): row = max(flat >> 7, 0)
            rowi = sb.tile([M, 24], I32)
            nc.vector.tensor_scalar(
                rowi[:], flat[:, 3:27], 7, None, Alu.arith_shift_right
            )
            idx16 = sb.tile([M, 24], I16)
            nc.vector.tensor_scalar(idx16[:], rowi[:], 0, None, Alu.max)

            # one dma_gather for all 384 rows: g[m, l, :] = tab[idx(m,l), :]
            g = sb.tile([M, NLVL, ROW], F32)
            nc.gpsimd.dma_gather(g[:], tab[:], idx16[:], NIDX, NIDX, ROW)

            # natural-layout remainder, validity, iota (overlap the gather)
            rem = sb.tile([M, 3], I32)
            nc.vector.tensor_scalar(
                rem[:], flat[:, 0:3], ROW - 1, None, Alu.bitwise_and
            )
            remf = sb.tile([M, 3], F32)
            nc.vector.tensor_copy(remf[:], rem[:])
            combo = sb.tile([M, 6], F32)
            validf = combo[:, 3:6]
            anchors = ti[:, 0:15].rearrange("p (l f) -> p l f", f=5)[:, :, 0:1]
            nc.vector.tensor_scalar(
                validf.rearrange("p (l f) -> p l f", f=1),
                anchors,
                -1,
                None,
                Alu.is_gt,
            )
            # pred[m,l] = g[m,l,rem[m,l]] -- fused (iota==rem)*g + row-sum
            pred = sb.tile([M, 3], F32)
            scratch = sb.tile([M, ROW], F32)
            for l in range(3):
                nc.vector.scalar_tensor_tensor(
                    out=scratch[:],
                    in0=iota,
                    scalar=remf[:, l : l + 1],
                    in1=g[:, l, :],
                    op0=Alu.is_equal,
                    op1=Alu.mult,
                    accum_out=pred[:, l : l + 1],
                )

            # smooth l1 (x2): d^2 - relu(|d|-1)^2   (the 0.5 lives in wk)
            d = sb.tile([M, 3], F32)
            nc.vector.tensor_tensor(d[:], pred[:], gts, Alu.subtract)
            dd = sb.tile([M, 3], F32)
            nc.vector.tensor_tensor(dd[:], d[:], d[:], Alu.mult)
            nd = sb.tile([M, 3], F32)
            nc.vector.tensor_scalar(nd[:], d[:], -1.0, None, Alu.mult)
            ad = sb.tile([M, 3], F32)
            nc.vector.tensor_tensor(ad[:], d[:], nd[:], Alu.max)
            t = sb.tile([M, 3], F32)
            nc.vector.tensor_scalar(t[:], ad[:], 1.0, 0.0, Alu.subtract, Alu.max)
            tt2 = sb.tile([M, 3], F32)
            nc.vector.tensor_tensor(tt2[:], t[:], t[:], Alu.mult)
            sl2 = sb.tile([M, 3], F32)
            nc.vector.tensor_tensor(sl2[:], dd[:], tt2[:], Alu.subtract)

            # combo = [ sl2*valid | valid ] -> one matmul -> [1,6];
            # wk/wen and the per-level active mask applied on partition 0
            nc.vector.tensor_tensor(combo[:, 0:3], sl2[:], validf, Alu.mult)
            pt6 = ps.tile([1, 6], F32)
            nc.tensor.matmul(
                out=pt6[:], lhsT=onec, rhs=combo[:], start=True, stop=True
            )
            res6 = sb.tile([1, 6], F32)
            act_b = (
                combo[0:1, 3:6]
                .rearrange("p (a l) -> p a l", a=1)
                .broadcast_to([1, 2, 3])
            )
            nc.vector.tensor_tensor(
                res6[:].rearrange("p (a l) -> p a l", l=3),
                pt6[:].rearrange("p (a l) -> p a l", l=3),
                act_b,
                Alu.mult,
            )
            nc.sync.dma_start(out=out[:], in_=res6[:])
    nc.finalize()
    return nc


_NC = None


def _get_nc():
    global _NC
    if _NC is None:
        _NC = _build_bass()
    return _NC


_IOTA = np.tile(np.arange(ROW, dtype=np.float32), (M, 1))
_STRIDE5 = {
    g: np.array([g**3, g**2, g, 1, 1], dtype=np.int64) for g in GRIDS
}


def kernel(**inputs: np.ndarray):
    out_l = [np.asarray(inputs[n]) for n in ("out1", "out3", "out5")]
    # level l uses coord/diff (2-l)  (the reference pairs them reversed)
    coords = [np.asarray(inputs[f"coord{2 - l}"]) for l in range(3)]
    diffs = [np.asarray(inputs[f"diff{2 - l}"]) for l in range(3)]

    in_maps = []
    for c in range(N_CORES):
        b, k = c // 4, c % 4
        im = {}
        im["tab"] = np.concatenate(
            [
                np.ascontiguousarray(out_l[l][b, A * k : A * (k + 1)]).reshape(
                    V[l], ROW
                )
                for l in range(3)
            ],
            axis=0,
        )
        co = np.zeros((M, NCOORD), np.int32)
        for l, g in enumerate(GRIDS):
            s = _STRIDE5[g]
            cc = coords[l][b].astype(np.int64)  # [128, 4]
            # premultiplied terms; 5th term = concat-table element base
            p5 = np.concatenate(
                [cc * s[:4], np.full((M, 1), VBASE[l] * ROW, np.int64)], axis=1
            )
            co[:, l * 5 : (l + 1) * 5] = p5.astype(np.int32)  # natural
            w = (
                p5.astype(np.int32).reshape(8, 16, 5).transpose(1, 0, 2)
            ).reshape(16, 40)
            co[:, 15 + l * 40 : 15 + (l + 1) * 40] = np.tile(w, (8, 1))
        im["auxp"] = co
        gts = np.stack([diffs[l][b, :, k] for l in range(3)], axis=1)
        onesc = np.ones((M, 1), np.float32)
        im["auxf"] = np.concatenate([_IOTA, gts, onesc], axis=1).astype(np.float32)
        in_maps.append(im)

    global _last_in_maps
    _last_in_maps = in_maps
    res = run_bass_kernel_spmd(_get_nc(), in_maps, core_ids=list(range(N_CORES)))
    # host epilogue of the reduction: per-core constant loss-weight scaling
    # (0.5*LOSS_W[k], weight counted once via the k==0 cores) + all-reduce
    loss = np.float32(0.0)
    weight = np.float32(0.0)
    for c in range(N_CORES):
        k = c % 4
        p6 = res.results[c]["partial"][0]
        loss += np.float32(p6[0:3].sum() * np.float32(0.5 * LOSS_W[k]))
        if k == 0:
            weight += np.float32(p6[3:6].sum())
    return (np.array([loss], np.float32), np.array([weight], np.float32))


# revision 22
# speedup vs baseline: 1.0021x; 1.0021x over previous
"""Bbox regression loss (smooth-L1 over gathered bbox deltas) on 8 TRN2 cores.

The loss gathers 4 scalars per (batch, gt-box) from each FPN level's dense
prediction tensor, applies smooth-L1 against the gt deltas, and reduces to
two scalars (weighted loss sum, valid-box count).  Only 3 x 2 x 128 x 4 =
3072 elements of the ~92MB of predictions are ever read, so the kernel is
built around one on-device dma_gather rather than streaming.

Sharding: core c handles (b = c//4, k = c%4) where k indexes the 4 bbox
coordinate channels (channel group k*A:(k+1)*A of the 4*A=12 channel dim).
Each core receives exactly 1/8 of every prediction tensor (concatenated
into one row table), computes its partial (loss, weight) fully on device,
and the host sums the 8 partials.

Device pipeline per core:
  1. coord math -> 512B-row indices for all 3 levels in the
     16-partition-wrapped int16 layout dma_gather wants.  The host ships
     stride-premultiplied coordinate terms (incl. the concat-table row
     base as a 5th term), so the device only clamps, 5-term-reduces and
     shifts.  Clamping the premultiplied anchor term is exact because only
     the anchor carries the -1 sentinel and the strides are positive.
  2. one dma_gather fetches the 384 rows -> [128(m), 3(level), 128] f32
  3. fused scalar_tensor_tensor one-hot select (iota==rem)*g with
     per-partition accumulate -> pred[m,l]
  4. smooth-L1 via the huber identity 0.5*(d^2 - relu(|d|-1)^2) (the 0.5
     folded into the host-side weight), validity masking, one matmul
     partition-reduction, per-level active mask applied on partition 0
"""

import os

import numpy as np

try:  # persistent XLA/NEFF compile cache across processes
    import jax

    os.makedirs("/tmp/jax_pcache", exist_ok=True)
    jax.config.update("jax_compilation_cache_dir", "/tmp/jax_pcache")
    jax.config.update("jax_persistent_cache_min_compile_time_secs", 0.0)
    jax.config.update("jax_persistent_cache_min_entry_size_bytes", 0)
except Exception:
    pass

import concourse.bacc as bacc
import concourse.bass as bass
import concourse.tile as tile
from concourse import mybir
from concourse.bass_utils import run_bass_kernel_spmd

A = 3                       # anchors per level
M = 128                     # gt entries per sample
GRIDS = (96, 48, 24)        # level l grid; level l uses coord/diff index 2-l
LOSS_W = (1.0, 1.0, 1.0, 0.1)
ROW = 128                   # f32 elements per gather row (512B)
NLVL = 3
NIDX = NLVL * M             # 384 gathered rows per core
V = tuple(A * g * g * g // ROW for g in GRIDS)      # (20736, 2592, 324)
VBASE = (0, V[0], V[0] + V[1])
VTOT = sum(V)               # 23652 rows < int16 max
N_CORES = 8

F32 = mybir.dt.float32
I32 = mybir.dt.int32
I16 = mybir.dt.int16
Alu = mybir.AluOpType

# auxw (int32, [128, 120]): stride-premultiplied coord terms, 24 groups x 5,
#   wrapped layout (l,q): partition = m%16, m = q*16+p%16,
#   5th term = VBASE[l]*ROW (element base of level l's rows).
# auxn (int32, [128, 15]): same terms in natural layout (partition = m),
#   3 groups x 5, 5th term = VBASE[l]*ROW (cancels in the &127 remainder).
NWRAP = 128  # 120 used + 8 pad cols to reach 512B/partition (full-rate DMA)
NNAT = 15
# auxf (f32, [128, 132]): iota128 | gts(3) | ones
NF_COLS = ROW + 4


def _build_bass() -> bass.Bass:
    nc = bacc.Bacc(
        "TRN2", target_bir_lowering=False, debug=False, num_devices=N_CORES
    )
    tab = nc.dram_tensor("tab", [VTOT, ROW], F32, kind="ExternalInput")
    auxw = nc.dram_tensor("auxw", [M, NWRAP], I32, kind="ExternalInput")
    auxn = nc.dram_tensor("auxn", [M, NNAT], I32, kind="ExternalInput")
    auxf = nc.dram_tensor("auxf", [M, NF_COLS], F32, kind="ExternalInput")
    out = nc.dram_tensor("partial", [1, 6], F32, kind="ExternalOutput")

    with tile.TileContext(nc) as tc:
        with (
            tc.tile_pool(name="sb", bufs=1) as sb,
            tc.tile_pool(name="ps", bufs=1, space="PSUM") as ps,
        ):
            tw = sb.tile([M, NWRAP], I32)
            nc.sync.dma_start(out=tw[:], in_=auxw[:])
            tn = sb.tile([M, NNAT], I32)
            nc.sync.dma_start(out=tn[:], in_=auxn[:])
            tf = sb.tile([M, NF_COLS], F32)
            nc.sync.dma_start(out=tf[:], in_=auxf[:])
            iota = tf[:, 0:ROW]
            gts = tf[:, ROW : ROW + 3]
            onec = tf[:, ROW + 3 : ROW + 4]

            # flatw[., c] = sum of premultiplied terms (no clamp: only the
            # anchor term of invalid entries is negative; the row index is
            # clamped at 0 below and those entries are masked)
            flatw = sb.tile([M, 24], I32)
            with nc.allow_low_precision(reason="exact int32 index arithmetic"):
                nc.vector.tensor_reduce(
                    flatw[:],
                    tw[:, 0:120].rearrange("p (c f) -> p c f", f=5),
                    axis=mybir.AxisListType.X,
                    op=Alu.add,
                )

            # wrapped row indices (int16): row = max(flatw >> 7, 0)
            rowi = sb.tile([M, 24], I32)
            nc.vector.tensor_scalar(
                rowi[:], flatw[:], 7, None, Alu.arith_shift_right
            )
            idx16 = sb.tile([M, 24], I16)
            nc.vector.tensor_scalar(idx16[:], rowi[:], 0, None, Alu.max)

            # one dma_gather for all 384 rows: g[m, l, :] = tab[idx(m,l), :]
            g = sb.tile([M, NLVL, ROW], F32)
            nc.gpsimd.dma_gather(g[:], tab[:], idx16[:], NIDX, NIDX, ROW)

            # natural-layout remainder + validity (runs under the gather)
            flatn = sb.tile([M, 3], I32)
            with nc.allow_low_precision(reason="exact int32 index arithmetic"):
                nc.vector.tensor_reduce(
                    flatn[:],
                    tn[:].rearrange("p (c f) -> p c f", f=5),
                    axis=mybir.AxisListType.X,
                    op=Alu.add,
                )
            rem = sb.tile([M, 3], I32)
            nc.vector.tensor_scalar(
                rem[:], flatn[:], ROW - 1, None, Alu.bitwise_and
            )
            remf = sb.tile([M, 3], F32)
            nc.vector.tensor_copy(remf[:], rem[:])
            combo = sb.tile([M, 6], F32)
            validf = combo[:, 3:6]
            anchors = tn[:].rearrange("p (l f) -> p l f", f=5)[:, :, 0:1]
            nc.vector.tensor_scalar(
                validf.rearrange("p (l f) -> p l f", f=1),
                anchors,
                -1,
                None,
                Alu.is_gt,
            )
            # pred[m,l] = g[m,l,rem[m,l]] -- fused (iota==rem)*g + row-sum
            pred = sb.tile([M, 3], F32)
            scratch = sb.tile([M, ROW], F32)
            for l in range(3):
                nc.vector.scalar_tensor_tensor(
                    out=scratch[:],
                    in0=iota,
                    scalar=remf[:, l : l + 1],
                    in1=g[:, l, :],
                    op0=Alu.is_equal,
                    op1=Alu.mult,
                    accum_out=pred[:, l : l + 1],
                )

            # smooth l1 (x2): d^2 - relu(|d|-1)^2   (the 0.5 lives in wk)
            d = sb.tile([M, 3], F32)
            nc.vector.tensor_tensor(d[:], pred[:], gts, Alu.subtract)
            dd = sb.tile([M, 3], F32)
            nc.vector.tensor_tensor(dd[:], d[:], d[:], Alu.mult)
            nd = sb.tile([M, 3], F32)
            nc.vector.tensor_scalar(nd[:], d[:], -1.0, None, Alu.mult)
            ad = sb.tile([M, 3], F32)
            nc.vector.tensor_tensor(ad[:], d[:], nd[:], Alu.max)
            t = sb.tile([M, 3], F32)
            nc.vector.tensor_scalar(t[:], ad[:], 1.0, 0.0, Alu.subtract, Alu.max)
            tt2 = sb.tile([M, 3], F32)
            nc.vector.tensor_tensor(tt2[:], t[:], t[:], Alu.mult)
            sl2 = sb.tile([M, 3], F32)
            nc.vector.tensor_tensor(sl2[:], dd[:], tt2[:], Alu.subtract)

            # combo = [ sl2*valid | valid ] -> one matmul -> [1,6];
            # wk/wen and the per-level active mask applied on partition 0
            nc.vector.tensor_tensor(combo[:, 0:3], sl2[:], validf, Alu.mult)
            pt6 = ps.tile([1, 6], F32)
            nc.tensor.matmul(
                out=pt6[:], lhsT=onec, rhs=combo[:], start=True, stop=True
            )
            res6 = sb.tile([1, 6], F32)
            act_b = (
                combo[0:1, 3:6]
                .rearrange("p (a l) -> p a l", a=1)
                .broadcast_to([1, 2, 3])
            )
            nc.vector.tensor_tensor(
                res6[:].rearrange("p (a l) -> p a l", l=3),
                pt6[:].rearrange("p (a l) -> p a l", l=3),
                act_b,
                Alu.mult,
            )
            nc.sync.dma_start(out=out[:], in_=res6[:])
    nc.finalize()
    return nc


_NC = None


def _get_nc():
    global _NC
    if _NC is None:
        _NC = _build_bass()
    return _NC


_IOTA = np.tile(np.arange(ROW, dtype=np.float32), (M, 1))
_STRIDE5 = {
    g: np.array([g**3, g**2, g, 1, 1], dtype=np.int64) for g in GRIDS
}


def kernel(**inputs: np.ndarray):
    out_l = [np.asarray(inputs[n]) for n in ("out1", "out3", "out5")]
    # level l uses coord/diff (2-l)  (the reference pairs them reversed)
    coords = [np.asarray(inputs[f"coord{2 - l}"]) for l in range(3)]
    diffs = [np.asarray(inputs[f"diff{2 - l}"]) for l in range(3)]

    in_maps = []
    for c in range(N_CORES):
        b, k = c // 4, c % 4
        im = {}
        im["tab"] = np.concatenate(
            [
                np.ascontiguousarray(out_l[l][b, A * k : A * (k + 1)]).reshape(
                    V[l], ROW
                )
                for l in range(3)
            ],
            axis=0,
        )
        cow = np.zeros((M, NWRAP), np.int32)
        con = np.zeros((M, NNAT), np.int32)
        for l, g in enumerate(GRIDS):
            s = _STRIDE5[g]
            cc = coords[l][b].astype(np.int64)  # [128, 4]
            # premultiplied terms; 5th term = concat-table element base
            p5 = np.concatenate(
                [cc * s[:4], np.full((M, 1), VBASE[l] * ROW, np.int64)], axis=1
            )
            con[:, l * 5 : (l + 1) * 5] = p5.astype(np.int32)
            w = (
                p5.astype(np.int32).reshape(8, 16, 5).transpose(1, 0, 2)
            ).reshape(16, 40)
            cow[:, l * 40 : (l + 1) * 40] = np.tile(w, (8, 1))
        im["auxw"] = cow
        im["auxn"] = con
        gts = np.stack([diffs[l][b, :, k] for l in range(3)], axis=1)
        onesc = np.ones((M, 1), np.float32)
        im["auxf"] = np.concatenate([_IOTA, gts, onesc], axis=1).astype(np.float32)
        in_maps.append(im)

    global _last_in_maps
    _last_in_maps = in_maps
    res = run_bass_kernel_spmd(_get_nc(), in_maps, core_ids=list(range(N_CORES)))
    # host epilogue of the reduction: per-core constant loss-weight scaling
    # (0.5*LOSS_W[k], weight counted once via the k==0 cores) + all-reduce
    loss = np.float32(0.0)
    weight = np.float32(0.0)
    for c in range(N_CORES):
        k = c % 4
        p6 = res.results[c]["partial"][0]
        loss += np.float32(p6[0:3].sum() * np.float32(0.5 * LOSS_W[k]))
        if k == 0:
            weight += np.float32(p6[3:6].sum())
    return (np.array([loss], np.float32), np.array([weight], np.float32))


# revision 24
# speedup vs baseline: 1.0042x; 1.0020x over previous
"""Bbox regression loss (smooth-L1 over gathered bbox deltas) on 8 TRN2 cores.

The loss gathers 4 scalars per (batch, gt-box) from each FPN level's dense
prediction tensor, applies smooth-L1 against the gt deltas, and reduces to
two scalars (weighted loss sum, valid-box count).  Only 3 x 2 x 128 x 4 =
3072 elements of the ~92MB of predictions are ever read, so the kernel is
built around one on-device dma_gather rather than streaming.

Sharding: core c handles (b = c//4, k = c%4) where k indexes the 4 bbox
coordinate channels (channel group k*A:(k+1)*A of the 4*A=12 channel dim).
Each core receives exactly 1/8 of every prediction tensor (concatenated
into one row table), computes its partial (loss, weight) fully on device,
and the host sums the 8 partials.

Device pipeline per core:
  1. coord math -> 512B-row indices for all 3 levels in the
     16-partition-wrapped int16 layout dma_gather wants.  The host ships
     stride-premultiplied coordinate terms (incl. the concat-table row
     base as a 5th term), so the device only clamps, 5-term-reduces and
     shifts.  Clamping the premultiplied anchor term is exact because only
     the anchor carries the -1 sentinel and the strides are positive.
  2. one dma_gather fetches the 384 rows -> [128(m), 3(level), 128] f32
  3. fused scalar_tensor_tensor one-hot select (iota==rem)*g with
     per-partition accumulate -> pred[m,l]
  4. smooth-L1 via the huber identity 0.5*(d^2 - relu(|d|-1)^2) (the 0.5
     folded into the host-side weight), validity masking, one matmul
     partition-reduction, per-level active mask applied on partition 0
"""

import os

import numpy as np

try:  # persistent XLA/NEFF compile cache across processes
    import jax

    os.makedirs("/tmp/jax_pcache", exist_ok=True)
    jax.config.update("jax_compilation_cache_dir", "/tmp/jax_pcache")
    jax.config.update("jax_persistent_cache_min_compile_time_secs", 0.0)
    jax.config.update("jax_persistent_cache_min_entry_size_bytes", 0)
except Exception:
    pass

import concourse.bacc as bacc
import concourse.bass as bass
import concourse.tile as tile
from concourse import mybir
from concourse.bass_utils import run_bass_kernel_spmd

A = 3                       # anchors per level
M = 128                     # gt entries per sample
GRIDS = (96, 48, 24)        # level l grid; level l uses coord/diff index 2-l
LOSS_W = (1.0, 1.0, 1.0, 0.1)
ROW = 128                   # f32 elements per gather row (512B)
NLVL = 3
NIDX = NLVL * M             # 384 gathered rows per core
V = tuple(A * g * g * g // ROW for g in GRIDS)      # (20736, 2592, 324)
VBASE = (0, V[0], V[0] + V[1])
VTOT = sum(V)               # 23652 rows < int16 max
N_CORES = 8

F32 = mybir.dt.float32
I32 = mybir.dt.int32
I16 = mybir.dt.int16
Alu = mybir.AluOpType

# auxw (int32, [128, 128]): stride-premultiplied coord terms, 24 groups x 4
#   [max(a,0)*g^3, d*g^2, h*g, w + VBASE[l]*ROW], wrapped layout (l,q):
#   partition = m%16, m = q*16+p%16.  The anchor clamp (the reference's
#   jnp.maximum(c,0) gather guard) is applied host-side here; validity is
#   still derived on device from the unclamped natural-layout anchor term.
# auxn (int32, [128, 12]): same 4 terms, natural layout (partition = m),
#   anchor term UNclamped (carries the -1 sentinel for the validity test).
NWRAP = 128  # 96 used + 32 pad cols to reach 512B/partition (full-rate DMA)
NNAT = 12
# auxf (f32, [128, 132]): iota128 | gts(3) | ones
NF_COLS = ROW + 4


def _build_bass() -> bass.Bass:
    nc = bacc.Bacc(
        "TRN2", target_bir_lowering=False, debug=False, num_devices=N_CORES
    )
    tab = nc.dram_tensor("tab", [VTOT, ROW], F32, kind="ExternalInput")
    auxw = nc.dram_tensor("auxw", [M, NWRAP], I32, kind="ExternalInput")
    auxn = nc.dram_tensor("auxn", [M, NNAT], I32, kind="ExternalInput")
    auxf = nc.dram_tensor("auxf", [M, NF_COLS], F32, kind="ExternalInput")
    out = nc.dram_tensor("partial", [1, 6], F32, kind="ExternalOutput")

    with tile.TileContext(nc) as tc:
        with (
            tc.tile_pool(name="sb", bufs=1) as sb,
            tc.tile_pool(name="ps", bufs=1, space="PSUM") as ps,
        ):
            tw = sb.tile([M, NWRAP], I32)
            nc.sync.dma_start(out=tw[:], in_=auxw[:])
            tn = sb.tile([M, NNAT], I32)
            nc.sync.dma_start(out=tn[:], in_=auxn[:])
            tf = sb.tile([M, NF_COLS], F32)
            nc.sync.dma_start(out=tf[:], in_=auxf[:])
            iota = tf[:, 0:ROW]
            gts = tf[:, ROW : ROW + 3]
            onec = tf[:, ROW + 3 : ROW + 4]

            # flatw[., c] = sum of premultiplied terms (anchor pre-clamped)
            flatw = sb.tile([M, 24], I32)
            with nc.allow_low_precision(reason="exact int32 index arithmetic"):
                nc.vector.tensor_reduce(
                    flatw[:],
                    tw[:, 0:96].rearrange("p (c f) -> p c f", f=4),
                    axis=mybir.AxisListType.X,
                    op=Alu.add,
                )

            # wrapped row indices: row = flatw >> 7 (bitwise can't cast;
            # the max-0 no-op does the int32->int16 conversion)
            rowi = sb.tile([M, 24], I32)
            nc.vector.tensor_scalar(
                rowi[:], flatw[:], 7, None, Alu.arith_shift_right
            )
            idx16 = sb.tile([M, 24], I16)
            nc.vector.tensor_scalar(idx16[:], rowi[:], 0, None, Alu.max)

            # one dma_gather for all 384 rows: g[m, l, :] = tab[idx(m,l), :]
            g = sb.tile([M, NLVL, ROW], F32)
            nc.gpsimd.dma_gather(g[:], tab[:], idx16[:], NIDX, NIDX, ROW)

            # natural-layout remainder + validity (runs under the gather)
            flatn = sb.tile([M, 3], I32)
            with nc.allow_low_precision(reason="exact int32 index arithmetic"):
                nc.vector.tensor_reduce(
                    flatn[:],
                    tn[:].rearrange("p (c f) -> p c f", f=4),
                    axis=mybir.AxisListType.X,
                    op=Alu.add,
                )
            rem = sb.tile([M, 3], I32)
            nc.vector.tensor_scalar(
                rem[:], flatn[:], ROW - 1, None, Alu.bitwise_and
            )
            remf = sb.tile([M, 3], F32)
            nc.vector.tensor_copy(remf[:], rem[:])
            combo = sb.tile([M, 6], F32)
            validf = combo[:, 3:6]
            anchors = tn[:].rearrange("p (l f) -> p l f", f=4)[:, :, 0:1]
            nc.vector.tensor_scalar(
                validf.rearrange("p (l f) -> p l f", f=1),
                anchors,
                -1,
                None,
                Alu.is_gt,
            )
            # pred[m,l] = g[m,l,rem[m,l]] -- fused (iota==rem)*g + row-sum
            pred = sb.tile([M, 3], F32)
            scratch = sb.tile([M, ROW], F32)
            for l in range(3):
                nc.vector.scalar_tensor_tensor(
                    out=scratch[:],
                    in0=iota,
                    scalar=remf[:, l : l + 1],
                    in1=g[:, l, :],
                    op0=Alu.is_equal,
                    op1=Alu.mult,
                    accum_out=pred[:, l : l + 1],
                )

            # smooth l1 (x2): d^2 - relu(|d|-1)^2   (the 0.5 lives in wk)
            d = sb.tile([M, 3], F32)
            nc.vector.tensor_tensor(d[:], pred[:], gts, Alu.subtract)
            dd = sb.tile([M, 3], F32)
            nc.vector.tensor_tensor(dd[:], d[:], d[:], Alu.mult)
            nd = sb.tile([M, 3], F32)
            nc.vector.tensor_scalar(nd[:], d[:], -1.0, None, Alu.mult)
            ad = sb.tile([M, 3], F32)
            nc.vector.tensor_tensor(ad[:], d[:], nd[:], Alu.max)
            t = sb.tile([M, 3], F32)
            nc.vector.tensor_scalar(t[:], ad[:], 1.0, 0.0, Alu.subtract, Alu.max)
            tt2 = sb.tile([M, 3], F32)
            nc.vector.tensor_tensor(tt2[:], t[:], t[:], Alu.mult)
            sl2 = sb.tile([M, 3], F32)
            nc.vector.tensor_tensor(sl2[:], dd[:], tt2[:], Alu.subtract)

            # combo = [ sl2*valid | valid ] -> one matmul -> [1,6];
            # wk/wen and the per-level active mask applied on partition 0
            nc.vector.tensor_tensor(combo[:, 0:3], sl2[:], validf, Alu.mult)
            pt6 = ps.tile([1, 6], F32)
            nc.tensor.matmul(
                out=pt6[:], lhsT=onec, rhs=combo[:], start=True, stop=True
            )
            res6 = sb.tile([1, 6], F32)
            act_b = (
                combo[0:1, 3:6]
                .rearrange("p (a l) -> p a l", a=1)
                .broadcast_to([1, 2, 3])
            )
            nc.vector.tensor_tensor(
                res6[:].rearrange("p (a l) -> p a l", l=3),
                pt6[:].rearrange("p (a l) -> p a l", l=3),
                act_b,
                Alu.mult,
            )
            nc.sync.dma_start(out=out[:], in_=res6[:])
    nc.finalize()
    return nc


_NC = None


def _get_nc():
    global _NC
    if _NC is None:
        _NC = _build_bass()
    return _NC


_IOTA = np.tile(np.arange(ROW, dtype=np.float32), (M, 1))
_STRIDE5 = {
    g: np.array([g**3, g**2, g, 1, 1], dtype=np.int64) for g in GRIDS
}


def kernel(**inputs: np.ndarray):
    out_l = [np.asarray(inputs[n]) for n in ("out1", "out3", "out5")]
    # level l uses coord/diff (2-l)  (the reference pairs them reversed)
    coords = [np.asarray(inputs[f"coord{2 - l}"]) for l in range(3)]
    diffs = [np.asarray(inputs[f"diff{2 - l}"]) for l in range(3)]

    in_maps = []
    for c in range(N_CORES):
        b, k = c // 4, c % 4
        im = {}
        im["tab"] = np.concatenate(
            [
                np.ascontiguousarray(out_l[l][b, A * k : A * (k + 1)]).reshape(
                    V[l], ROW
                )
                for l in range(3)
            ],
            axis=0,
        )
        cow = np.zeros((M, NWRAP), np.int32)
        con = np.zeros((M, NNAT), np.int32)
        for l, g in enumerate(GRIDS):
            s = _STRIDE5[g]
            cc = coords[l][b].astype(np.int64)  # [128, 4]
            # 4 premultiplied terms; table row base folded into the w term
            p4 = cc * s[:4]
            p4[:, 3] += VBASE[l] * ROW
            con[:, l * 4 : (l + 1) * 4] = p4.astype(np.int32)
            p4c = p4.copy()
            p4c[:, 0] = np.maximum(cc[:, 0], 0) * s[0]  # anchor gather clamp
            w = (
                p4c.astype(np.int32).reshape(8, 16, 4).transpose(1, 0, 2)
            ).reshape(16, 32)
            cow[:, l * 32 : (l + 1) * 32] = np.tile(w, (8, 1))
        im["auxw"] = cow
        im["auxn"] = con
        gts = np.stack([diffs[l][b, :, k] for l in range(3)], axis=1)
        onesc = np.ones((M, 1), np.float32)
        im["auxf"] = np.concatenate([_IOTA, gts, onesc], axis=1).astype(np.float32)
        in_maps.append(im)

    global _last_in_maps
    _last_in_maps = in_maps
    res = run_bass_kernel_spmd(_get_nc(), in_maps, core_ids=list(range(N_CORES)))
    # host epilogue of the reduction: per-core constant loss-weight scaling
    # (0.5*LOSS_W[k], weight counted once via the k==0 cores) + all-reduce
    loss = np.float32(0.0)
    weight = np.float32(0.0)
    for c in range(N_CORES):
        k = c % 4
        p6 = res.results[c]["partial"][0]
        loss += np.float32(p6[0:3].sum() * np.float32(0.5 * LOSS_W[k]))
        if k == 0:
            weight += np.float32(p6[3:6].sum())
    return (np.array([loss], np.float32), np.array([weight], np.float32))
